# revision 4
# baseline (speedup 1.0000x reference)
"""Bass/Trainium2 kernel for blockwise cross-attention.

Math (per batch element b, per 16-row block):
  out1 = softmax(q1 k2^T / sqrt(E)) @ v2,  out2 = softmax(q2 k1^T / sqrt(E)) @ v1
with q = x Wq^T + bq etc.  Since softmax is shift-invariant along the key
axis, the q-side bias terms drop and
  softmax(q1 k2^T / s) == softmax(x1 A x2^T + 1 (x2 c)^T)
with A = Wq^T Wk / s and c = Wk^T bq / s precomputed on the host.  This
replaces 6 big projections with 4 (z = x A^T fused for both q&k roles, plus
v' = x Wv^T).  The v bias folds in exactly because softmax rows sum to 1.

Sharding: pure data-parallel — batch B=8, one batch element per NeuronCore.

Device layout per core (S=4096 rows, E=512):
  - x^T tiles [128e, 512rows] per 512-row group (host pre-transposes, bf16)
  - z^T via stationary A^T chunks; t via stationary c chunks; v natural via
    stationary x^T chunks (+ ones-outer-product bias matmul)
  - scores window [128q,128k]: 4 e-chunk matmuls + rank-9 blockmask matmul
    (adds -100 off-block, exact 0 on-block, and could carry t) + 1x(t row)
  - softmax: ACT Exp with fused accum row-sum (off-block entries exp to ~0,
    so no explicit mask/max-subtract needed), DVE reciprocal + scale->bf16
  - attn^T: single DVE 32x32-block transpose == exact transpose of the
    block-diagonal attn (16-blocks lie inside diagonal 32-blocks; off-diag
    32-blocks are exactly 0 in bf16)
  - out = attnT.T @ v' single K=128 matmul -> PSUM -> copy -> DMA out fp32
"""

import math
import sys

if "/opt/trn_rl_repo" not in sys.path:
    sys.path.insert(0, "/opt/trn_rl_repo")

import numpy as np
import ml_dtypes

BF16 = ml_dtypes.bfloat16
MASK_C = 100.0  # off-block logit penalty; exp(x - 100) flushes to 0 in fp32
BLOCK = 16  # attention block size (ceil(S**(2/3)) blocks => 16 for S=4096)


def _build_nc(S: int, E: int):
    from contextlib import ExitStack

    import concourse.bass as bass
    import concourse.tile as tile
    from concourse import bacc, mybir

    f32 = mybir.dt.float32
    bf16 = mybir.dt.bfloat16
    P = 128
    GROUP = 512  # rows per group
    G = S // GROUP
    NCH = E // P  # e-chunks (4)
    NW = GROUP // P  # windows per group (4)
    assert S % GROUP == 0 and E == 512

    nc = bacc.Bacc("TRN2", debug=False)

    x_dram = [
        nc.dram_tensor("x1t", [E, S], bf16, kind="ExternalInput").ap(),
        nc.dram_tensor("x2t", [E, S], bf16, kind="ExternalInput").ap(),
    ]
    at_dram = nc.dram_tensor("at", [E, E], bf16, kind="ExternalInput").ap()
    wvt_dram = nc.dram_tensor("wvt", [E, E], bf16, kind="ExternalInput").ap()
    cv_dram = nc.dram_tensor("cvec", [P, NCH], bf16, kind="ExternalInput").ap()
    ones_dram = nc.dram_tensor("ones", [1, P], bf16, kind="ExternalInput").ap()
    bv_dram = nc.dram_tensor("bvrow", [1, E], bf16, kind="ExternalInput").ap()
    ml_dram = nc.dram_tensor("mlhs", [16, P], bf16, kind="ExternalInput").ap()
    mr_dram = nc.dram_tensor("mrhs", [16, P], bf16, kind="ExternalInput").ap()
    out_dram = [
        nc.dram_tensor("out1", [S, E], f32, kind="ExternalOutput").ap(),
        nc.dram_tensor("out2", [S, E], f32, kind="ExternalOutput").ap(),
    ]

    Exp = mybir.ActivationFunctionType.Exp

    with ExitStack() as ctx:
        tc = ctx.enter_context(tile.TileContext(nc))

        consts = ctx.enter_context(tc.tile_pool(name="consts", bufs=1))
        xt_pool = ctx.enter_context(tc.tile_pool(name="xt", bufs=2))
        z_pool = ctx.enter_context(tc.tile_pool(name="z", bufs=2))
        v_pool = ctx.enter_context(tc.tile_pool(name="v", bufs=2))
        t_pool = ctx.enter_context(tc.tile_pool(name="t", bufs=2))
        sm_pool = ctx.enter_context(tc.tile_pool(name="sm", bufs=3))
        o_pool = ctx.enter_context(tc.tile_pool(name="o", bufs=3))
        psA = ctx.enter_context(tc.tile_pool(name="psA", bufs=3, space="PSUM"))
        psS = ctx.enter_context(tc.tile_pool(name="psS", bufs=3, space="PSUM"))
        psO = ctx.enter_context(tc.tile_pool(name="psO", bufs=2, space="PSUM"))

        # --- constants ---
        at_t = []
        wv_t = []
        for c in range(NCH):
            a_tl = consts.tile([P, E], bf16, name=f"at{c}", tag=f"at{c}")
            nc.sync.dma_start(a_tl[:], at_dram[c * P : (c + 1) * P, :])
            at_t.append(a_tl)
            w_tl = consts.tile([P, E], bf16, name=f"wv{c}", tag=f"wv{c}")
            nc.sync.dma_start(w_tl[:], wvt_dram[c * P : (c + 1) * P, :])
            wv_t.append(w_tl)
        cv_t = consts.tile([P, NCH], bf16, name="cv", tag="cv")
        nc.sync.dma_start(cv_t[:], cv_dram[:])
        ones_t = consts.tile([1, P], bf16, name="onesr", tag="onesr")
        nc.sync.dma_start(ones_t[:], ones_dram[:])
        bv_t = consts.tile([1, E], bf16, name="bvr", tag="bvr")
        nc.sync.dma_start(bv_t[:], bv_dram[:])
        ml_t = consts.tile([16, P], bf16, name="mlh", tag="mlh")
        nc.sync.dma_start(ml_t[:], ml_dram[:])
        mr_t = consts.tile([16, P], bf16, name="mrh", tag="mrh")
        nc.sync.dma_start(mr_t[:], mr_dram[:])

        # --- main loop over 512-row groups ---
        for g in range(G):
            r0 = g * GROUP
            xt = {}
            zt = {}
            vt = {}
            tt = {}
            for s in range(2):
                for c in range(NCH):
                    x_tl = xt_pool.tile([P, GROUP], bf16, name=f"xt{s}{c}", tag=f"xt{s}{c}")
                    nc.sync.dma_start(x_tl[:], x_dram[s][c * P : (c + 1) * P, r0 : r0 + GROUP])
                    xt[s, c] = x_tl

            for s in range(2):
                # t_s[row] = sum_e x_s[row, e] * cvec[e]   -> [1, GROUP]
                t_ps = psA.tile([1, GROUP], f32, name="tps", tag="psA")
                for c in range(NCH):
                    nc.tensor.matmul(
                        t_ps[:], cv_t[:, c : c + 1], xt[s, c][:],
                        start=(c == 0), stop=(c == NCH - 1),
                    )
                t_sb = t_pool.tile([1, GROUP], bf16, name=f"tsb{s}", tag=f"tsb{s}")
                nc.vector.tensor_copy(t_sb[:], t_ps[:])
                tt[s] = t_sb

                # z_s^T m-chunk [128 e_out, GROUP rows]
                for m in range(NCH):
                    z_ps = psA.tile([P, GROUP], f32, name="zps", tag="psA")
                    for c in range(NCH):
                        nc.tensor.matmul(
                            z_ps[:], at_t[c][:, m * P : (m + 1) * P], xt[s, c][:],
                            start=(c == 0), stop=(c == NCH - 1),
                        )
                    z_sb = z_pool.tile([P, GROUP], bf16, name=f"zsb{s}{m}", tag=f"zsb{s}{m}")
                    nc.vector.tensor_copy(z_sb[:], z_ps[:])
                    zt[s, m] = z_sb

                # v'_s r-chunk [128 rows, E] = x @ Wv^T + 1 (x) bv
                for r in range(NW):
                    v_ps = psA.tile([P, E], f32, name="vps", tag="psA")
                    for c in range(NCH):
                        nc.tensor.matmul(
                            v_ps[:], xt[s, c][:, r * P : (r + 1) * P], wv_t[c][:],
                            start=(c == 0), stop=False,
                        )
                    nc.tensor.matmul(v_ps[:], ones_t[:], bv_t[:], start=False, stop=True)
                    v_sb = v_pool.tile([P, E], bf16, name=f"vsb{s}{r}", tag=f"vsb{s}{r}")
                    nc.scalar.copy(v_sb[:], v_ps[:])
                    vt[s, r] = v_sb

            # --- attention windows ---
            for w in range(NW):
                ws = slice(w * P, (w + 1) * P)
                for qs, ks in ((0, 1), (1, 0)):
                    s_ps = psS.tile([P, P], f32, name="sps", tag="psS")
                    for m in range(NCH):
                        nc.tensor.matmul(
                            s_ps[:], xt[qs, m][:, ws], zt[ks, m][:, ws],
                            start=(m == 0), stop=False,
                        )
                    # block mask: -C off-block, 0 on-block (rank 9)
                    nc.tensor.matmul(s_ps[:], ml_t[:], mr_t[:], start=False, stop=False)
                    # + t_ks[k] broadcast over q rows (rank 1)
                    nc.tensor.matmul(s_ps[:], ones_t[:], tt[ks][:, ws], start=False, stop=True)

                    exp_sb = sm_pool.tile([P, P], f32, name="expsb", tag="expsb")
                    rsum = sm_pool.tile([P, 1], f32, name="rsum", tag="rsum")
                    nc.scalar.activation(exp_sb[:], s_ps[:], Exp, accum_out=rsum[:])
                    rcp = sm_pool.tile([P, 1], f32, name="rcp", tag="rcp")
                    nc.vector.reciprocal(rcp[:], rsum[:])
                    attn = sm_pool.tile([P, P], bf16, name="attn", tag="attn")
                    nc.vector.tensor_scalar_mul(attn[:], exp_sb[:], rcp[:])
                    attnT = sm_pool.tile([P, P], bf16, name="attnT", tag="attnT")
                    nc.vector.transpose(attnT[:], attn[:])

                    o_ps = psO.tile([P, E], f32, name="ops", tag="psO")
                    nc.tensor.matmul(o_ps[:], attnT[:], vt[ks, w][:], start=True, stop=True)
                    o_sb = o_pool.tile([P, E], f32, name=f"osb{qs}", tag=f"osb{qs}")
                    if w % 2 == 0:
                        nc.scalar.copy(o_sb[:], o_ps[:])
                    else:
                        nc.vector.tensor_copy(o_sb[:], o_ps[:])
                    nc.gpsimd.dma_start(out_dram[qs][r0 + w * P : r0 + (w + 1) * P, :], o_sb[:])

    nc.compile()
    return nc


def _host_inputs(state1, state2, Wq, bq, Wk, bk, Wv, bv, S, E):
    """Build the per-core common (weight) arrays + per-core x arrays."""
    P = 128
    NCH = E // P
    scale = math.sqrt(E)
    Wq64 = np.asarray(Wq, np.float64)
    Wk64 = np.asarray(Wk, np.float64)
    # A = Wq^T Wk / scale ; device needs A^T = Wk^T Wq / scale  [e_in, e_out]
    at = (Wk64.T @ Wq64 / scale).astype(BF16)
    cvec = (Wk64.T @ np.asarray(bq, np.float64) / scale).astype(BF16)  # [E]
    wvt = np.ascontiguousarray(np.asarray(Wv, np.float32).T).astype(BF16)  # [e_in, e_out]
    cv2d = np.ascontiguousarray(cvec.reshape(NCH, P).T)  # [P, NCH], col c = chunk c
    ones = np.ones((1, P), BF16)
    bvrow = np.asarray(bv, np.float32).reshape(1, E).astype(BF16)
    # rank-9 block mask factors: sum_r mlhs[r,q]*mrhs[r,k] = -C + C*[same 16-block]
    mlhs = np.zeros((16, P), BF16)
    mrhs = np.zeros((16, P), BF16)
    q_idx = np.arange(P)
    mlhs[0, :] = 1.0
    mrhs[0, :] = np.float32(-MASK_C)
    for b in range(P // BLOCK):
        mlhs[1 + b, :] = (q_idx // BLOCK == b).astype(np.float32)
        mrhs[1 + b, :] = (MASK_C * (q_idx // BLOCK == b)).astype(np.float32)
    common = {
        "at": np.ascontiguousarray(at),
        "wvt": wvt,
        "cvec": cv2d,
        "ones": ones,
        "bvrow": bvrow,
        "mlhs": mlhs,
        "mrhs": mrhs,
    }
    x1 = np.asarray(state1, np.float32)
    x2 = np.asarray(state2, np.float32)
    B = x1.shape[0]
    per_core = []
    for b in range(B):
        per_core.append(
            {
                "x1t": np.ascontiguousarray(x1[b].T).astype(BF16),
                "x2t": np.ascontiguousarray(x2[b].T).astype(BF16),
                **common,
            }
        )
    return per_core


_NC_CACHE = {}


def _get_nc(S, E):
    key = (S, E)
    if key not in _NC_CACHE:
        _NC_CACHE[key] = _build_nc(S, E)
    return _NC_CACHE[key]


def kernel(state1, state2, Wq, bq, Wk, bk, Wv, bv):
    from concourse.bass_utils import run_bass_kernel_spmd

    state1 = np.asarray(state1)
    B, S, E = state1.shape
    assert (B, S, E) == (8, 4096, 512), (B, S, E)

    nc = _get_nc(S, E)
    in_maps = _host_inputs(state1, state2, Wq, bq, Wk, bk, Wv, bv, S, E)
    res = run_bass_kernel_spmd(nc, in_maps, list(range(B)))
    out1 = np.stack([res.results[b]["out1"] for b in range(B)])
    out2 = np.stack([res.results[b]["out2"] for b in range(B)])
    return out1, out2


if __name__ == "__main__":
    rng = np.random.default_rng(0)
    B, S, E = 8, 4096, 512
    ins = {
        "state1": rng.standard_normal((B, S, E), np.float32),
        "state2": rng.standard_normal((B, S, E), np.float32),
        "Wq": rng.standard_normal((E, E), np.float32) * 0.02,
        "bq": rng.standard_normal((E,), np.float32) * 0.02,
        "Wk": rng.standard_normal((E, E), np.float32) * 0.02,
        "bk": rng.standard_normal((E,), np.float32) * 0.02,
        "Wv": rng.standard_normal((E, E), np.float32) * 0.02,
        "bv": rng.standard_normal((E,), np.float32) * 0.02,
    }
    o1, o2 = kernel(**ins)
    print("ok", o1.shape, o2.shape, o1.dtype)


# revision 11
# speedup vs baseline: 1.1570x; 1.1570x over previous
"""Bass/Trainium2 kernel for blockwise cross-attention.

Math (per batch element b, per 16-row block):
  out1 = softmax(q1 k2^T / sqrt(E)) @ v2,  out2 = softmax(q2 k1^T / sqrt(E)) @ v1
with q = x Wq^T + bq etc.  Since softmax is shift-invariant along the key
axis, the q-side bias terms drop and
  softmax(q1 k2^T / s) == softmax(x1 A x2^T + 1 (x2 c)^T)
with A = Wq^T Wk / s and c = Wk^T bq / s precomputed on the host.  This
replaces 6 big projections with 4 (z = x A^T fused for both q&k roles, plus
v' = x Wv^T).  The v bias folds in exactly because softmax rows sum to 1.

Sharding: pure data-parallel — batch B=8, one batch element per NeuronCore.

Device flow per core (S=4096 rows, E=512), bf16 matmuls / fp32 softmax:
  - x^T tiles [128e, 512rows] per 512-row group (host pre-transposes, bf16)
  - z^T = A x^T via stationary A^T chunks; t = x c via stationary c chunks;
    v' = x Wv^T natural via stationary x^T chunks; v bias bv added during the
    PSUM->SBUF copy (DVE tensor_tensor with a broadcast bv tile)
  - scores window [128q,128k]: 4 e-chunk matmuls + one K=9 matmul that adds
    both the off-block -100 mask (rank 9: -100*1x1 + 100*sum u_b x u_b) and
    the key-side bias t[k] (folded into the rank-1 row as t[k]-100)
  - softmax: ACT Exp with fused accum row-sum (off-block entries exp to 0,
    so no explicit mask or max-subtraction is needed; logits are O(1)),
    DVE reciprocal + per-row scale -> bf16
  - attn^T: single DVE 32x32-block transpose == exact transpose of the
    block-diagonal attn (16-blocks lie inside diagonal 32-blocks; off-diag
    32-blocks are exactly 0)
  - out = attnT.T @ v' single K=128 matmul -> PSUM -> copy -> DMA out fp32
"""

import math
import sys

if "/opt/trn_rl_repo" not in sys.path:
    sys.path.insert(0, "/opt/trn_rl_repo")

import numpy as np
import ml_dtypes

BF16 = ml_dtypes.bfloat16
MASK_C = 100.0  # off-block logit penalty; exp(x - 100) flushes to 0 in fp32
BLOCK = 16  # attention block size (ceil(S**(2/3)) blocks => 16 for S=4096)


def _build_nc(S: int, E: int):
    from contextlib import ExitStack

    import concourse.bass as bass
    import concourse.tile as tile
    from concourse import bacc, mybir

    f32 = mybir.dt.float32
    bf16 = mybir.dt.bfloat16
    P = 128
    GROUP = 512  # rows per group
    G = S // GROUP
    NCH = E // P  # e-chunks (4)
    NW = GROUP // P  # windows per group (4)
    NB = P // BLOCK  # 16-blocks per window (8)
    assert S % GROUP == 0 and E == 512

    nc = bacc.Bacc("TRN2", debug=False)

    x_dram = [
        nc.dram_tensor("x1t", [E, S], bf16, kind="ExternalInput").ap(),
        nc.dram_tensor("x2t", [E, S], bf16, kind="ExternalInput").ap(),
    ]
    at_dram = nc.dram_tensor("at", [E, E], bf16, kind="ExternalInput").ap()
    wvt_dram = nc.dram_tensor("wvt", [E, E], bf16, kind="ExternalInput").ap()
    cv_dram = nc.dram_tensor("cvec", [P, NCH], bf16, kind="ExternalInput").ap()
    ml9_dram = nc.dram_tensor("ml9", [2 + NB, P], bf16, kind="ExternalInput").ap()
    m8r_dram = nc.dram_tensor("m8r", [1 + NB, GROUP], bf16, kind="ExternalInput").ap()
    bvb_dram = nc.dram_tensor("bvb", [P, E], f32, kind="ExternalInput").ap()
    out_dram = [
        nc.dram_tensor("out1", [S, E], f32, kind="ExternalOutput").ap(),
        nc.dram_tensor("out2", [S, E], f32, kind="ExternalOutput").ap(),
    ]

    Exp = mybir.ActivationFunctionType.Exp

    with ExitStack() as ctx:
        tc = ctx.enter_context(tile.TileContext(nc))

        consts = ctx.enter_context(tc.tile_pool(name="consts", bufs=1))
        xt_pool = ctx.enter_context(tc.tile_pool(name="xt", bufs=2))
        z_pool = ctx.enter_context(tc.tile_pool(name="z", bufs=2))
        v_pool = ctx.enter_context(tc.tile_pool(name="v", bufs=2))
        r5_pool = ctx.enter_context(tc.tile_pool(name="r5", bufs=2))
        sm_pool = ctx.enter_context(tc.tile_pool(name="sm", bufs=3))
        o_pool = ctx.enter_context(tc.tile_pool(name="o", bufs=3))
        psA = ctx.enter_context(tc.tile_pool(name="psA", bufs=3, space="PSUM"))
        psS = ctx.enter_context(tc.tile_pool(name="psS", bufs=3, space="PSUM"))
        psO = ctx.enter_context(tc.tile_pool(name="psO", bufs=2, space="PSUM"))

        # --- constants (batched DMAs) ---
        at_t = consts.tile([P, NCH * E], bf16, name="att", tag="att")
        nc.sync.dma_start(
            at_t.rearrange("p (c e) -> p c e", c=NCH),
            at_dram.rearrange("(c p) e -> p c e", p=P),
        )
        wv_t = consts.tile([P, NCH * E], bf16, name="wvt", tag="wvt")
        nc.sync.dma_start(
            wv_t.rearrange("p (c e) -> p c e", c=NCH),
            wvt_dram.rearrange("(c p) e -> p c e", p=P),
        )
        cv_t = consts.tile([P, NCH], bf16, name="cv", tag="cv")
        nc.sync.dma_start(cv_t[:], cv_dram[:])
        ml9_t = consts.tile([2 + NB, P], bf16, name="ml9", tag="ml9")
        nc.sync.dma_start(ml9_t[:], ml9_dram[:])
        bvb_t = consts.tile([P, E], f32, name="bvb", tag="bvb")
        nc.sync.dma_start(bvb_t[:], bvb_dram[:])

        def at_c(c):  # A^T chunk c: [128 e_in, 512 e_out]
            return at_t[:, c * E : (c + 1) * E]

        def wv_c(c):
            return wv_t[:, c * E : (c + 1) * E]

        # --- main loop over 512-row groups ---
        for g in range(G):
            r0 = g * GROUP
            xt = {}
            zt = {}
            vt = {}
            r5 = {}
            for s in range(2):
                x_tl = xt_pool.tile([P, NCH * GROUP], bf16, name=f"xt{s}", tag=f"xt{s}")
                nc.sync.dma_start(
                    x_tl.rearrange("p (c r) -> p c r", c=NCH),
                    x_dram[s].rearrange("(c p) s -> p c s", p=P)[:, :, r0 : r0 + GROUP],
                )
                xt[s] = x_tl

            def xt_c(s, c):  # x^T chunk c: [128 e_in, 512 rows]
                return xt[s][:, c * GROUP : (c + 1) * GROUP]

            for s in range(2):
                # scores-bias rhs tile [10, GROUP]: row0 = t_s (small, keeps
                # full bf16 relative precision), row1 = -C, rows2..9 = C*u_b
                r5_tl = r5_pool.tile([2 + NB, GROUP], bf16, name=f"r5{s}", tag=f"r5{s}")
                nc.sync.dma_start(r5_tl[1 : 2 + NB, :], m8r_dram[:])
                # t_s[row] = sum_e x_s[row, e] * cvec[e]   -> [1, GROUP]
                t_ps = psA.tile([1, GROUP], f32, name="tps", tag="psA")
                for c in range(NCH):
                    nc.tensor.matmul(
                        t_ps[:], cv_t[:, c : c + 1], xt_c(s, c),
                        start=(c == 0), stop=(c == NCH - 1),
                    )
                nc.vector.tensor_copy(r5_tl[0:1, :], t_ps[:])
                r5[s] = r5_tl

                # z_s^T m-chunk [128 e_out, GROUP rows]
                for m in range(NCH):
                    z_ps = psA.tile([P, GROUP], f32, name="zps", tag="psA")
                    for c in range(NCH):
                        nc.tensor.matmul(
                            z_ps[:], at_c(c)[:, m * P : (m + 1) * P], xt_c(s, c),
                            start=(c == 0), stop=(c == NCH - 1),
                        )
                    z_sb = z_pool.tile([P, GROUP], bf16, name=f"zsb{s}{m}", tag=f"zsb{s}{m}")
                    nc.scalar.copy(z_sb[:], z_ps[:])
                    zt[s, m] = z_sb

                # v'_s r-chunk [128 rows, E] = x @ Wv^T ; + bv during copy
                for r in range(NW):
                    v_ps = psA.tile([P, E], f32, name="vps", tag="psA")
                    for c in range(NCH):
                        nc.tensor.matmul(
                            v_ps[:], xt_c(s, c)[:, r * P : (r + 1) * P], wv_c(c),
                            start=(c == 0), stop=(c == NCH - 1),
                        )
                    v_sb = v_pool.tile([P, E], bf16, name=f"vsb{s}{r}", tag=f"vsb{s}{r}")
                    nc.vector.tensor_add(v_sb[:], v_ps[:], bvb_t[:])
                    vt[s, r] = v_sb

            # --- attention windows ---
            for w in range(NW):
                ws = slice(w * P, (w + 1) * P)
                for qs, ks in ((0, 1), (1, 0)):
                    s_ps = psS.tile([P, P], f32, name="sps", tag="psS")
                    for m in range(NCH):
                        nc.tensor.matmul(
                            s_ps[:], xt_c(qs, m)[:, ws], zt[ks, m][:, ws],
                            start=(m == 0), stop=False,
                        )
                    # + mask (-C off-block) + t_ks[k]: rank-10, K=10 matmul
                    nc.tensor.matmul(s_ps[:], ml9_t[:], r5[ks][:, ws], start=False, stop=True)

                    exp_sb = sm_pool.tile([P, P], f32, name="expsb", tag="expsb")
                    rsum = sm_pool.tile([P, 1], f32, name="rsum", tag="rsum")
                    nc.scalar.activation(exp_sb[:], s_ps[:], Exp, accum_out=rsum[:])
                    rcp = sm_pool.tile([P, 1], f32, name="rcp", tag="rcp")
                    nc.vector.reciprocal(rcp[:], rsum[:])
                    attn = sm_pool.tile([P, P], bf16, name="attn", tag="attn")
                    nc.vector.tensor_scalar_mul(attn[:], exp_sb[:], rcp[:])
                    attnT = sm_pool.tile([P, P], bf16, name="attnT", tag="attnT")
                    nc.vector.transpose(attnT[:], attn[:])

                    o_ps = psO.tile([P, E], f32, name="ops", tag="psO")
                    nc.tensor.matmul(o_ps[:], attnT[:], vt[ks, w][:], start=True, stop=True)
                    o_sb = o_pool.tile([P, E], f32, name=f"osb{qs}", tag=f"osb{qs}")
                    if w % 2 == 0:
                        nc.scalar.copy(o_sb[:], o_ps[:])
                    else:
                        nc.vector.tensor_copy(o_sb[:], o_ps[:])
                    nc.gpsimd.dma_start(out_dram[qs][r0 + w * P : r0 + (w + 1) * P, :], o_sb[:])

    nc.compile()
    return nc


def _host_inputs(state1, state2, Wq, bq, Wk, bk, Wv, bv, S, E):
    """Build the per-core common (weight) arrays + per-core x arrays."""
    P = 128
    GROUP = 512
    NCH = E // P
    NB = P // BLOCK
    scale = math.sqrt(E)
    Wq64 = np.asarray(Wq, np.float64)
    Wk64 = np.asarray(Wk, np.float64)
    # A = Wq^T Wk / scale ; device needs A^T = Wk^T Wq / scale  [e_in, e_out]
    at = (Wk64.T @ Wq64 / scale).astype(BF16)
    cvec = (Wk64.T @ np.asarray(bq, np.float64) / scale).astype(BF16)  # [E]
    wvt = np.ascontiguousarray(np.asarray(Wv, np.float32).T).astype(BF16)
    cv2d = np.ascontiguousarray(cvec.reshape(NCH, P).T)  # [P, NCH], col c = chunk c
    # rank-10 factors (q-side lhsT ml9, k-side const rows m8r):
    #   row0: 1 (x) t[k]   (t written on device)
    #   row1: 1 (x) -C     ; rows 2..9: u_b (x) C*u_b
    idx = np.arange(P)
    ml9 = np.zeros((2 + NB, P), BF16)
    ml9[0, :] = 1.0
    ml9[1, :] = 1.0
    for b in range(NB):
        ml9[2 + b, :] = (idx // BLOCK == b).astype(np.float32)
    kidx = np.arange(GROUP) % P
    m8r = np.zeros((1 + NB, GROUP), BF16)
    m8r[0, :] = np.float32(-MASK_C)
    for b in range(NB):
        m8r[1 + b, :] = (MASK_C * (kidx // BLOCK == b)).astype(np.float32)
    bvb = np.broadcast_to(np.asarray(bv, np.float32).reshape(1, E), (P, E))
    common = {
        "at": np.ascontiguousarray(at),
        "wvt": wvt,
        "cvec": cv2d,
        "ml9": ml9,
        "m8r": m8r,
        "bvb": np.ascontiguousarray(bvb),
    }
    x1 = np.asarray(state1, np.float32)
    x2 = np.asarray(state2, np.float32)
    B = x1.shape[0]
    per_core = []
    for b in range(B):
        per_core.append(
            {
                "x1t": np.ascontiguousarray(x1[b].T).astype(BF16),
                "x2t": np.ascontiguousarray(x2[b].T).astype(BF16),
                **common,
            }
        )
    return per_core


_NC_CACHE = {}


def _get_nc(S, E):
    key = (S, E)
    if key not in _NC_CACHE:
        _NC_CACHE[key] = _build_nc(S, E)
    return _NC_CACHE[key]


def kernel(state1, state2, Wq, bq, Wk, bk, Wv, bv):
    from concourse.bass_utils import run_bass_kernel_spmd

    state1 = np.asarray(state1)
    B, S, E = state1.shape
    assert (B, S, E) == (8, 4096, 512), (B, S, E)

    nc = _get_nc(S, E)
    in_maps = _host_inputs(state1, state2, Wq, bq, Wk, bk, Wv, bv, S, E)
    res = run_bass_kernel_spmd(nc, in_maps, list(range(B)))
    out1 = np.stack([res.results[b]["out1"] for b in range(B)])
    out2 = np.stack([res.results[b]["out2"] for b in range(B)])
    return out1, out2


if __name__ == "__main__":
    rng = np.random.default_rng(0)
    B, S, E = 8, 4096, 512
    ins = {
        "state1": rng.standard_normal((B, S, E), np.float32),
        "state2": rng.standard_normal((B, S, E), np.float32),
        "Wq": rng.standard_normal((E, E), np.float32) * 0.02,
        "bq": rng.standard_normal((E,), np.float32) * 0.02,
        "Wk": rng.standard_normal((E, E), np.float32) * 0.02,
        "bk": rng.standard_normal((E,), np.float32) * 0.02,
        "Wv": rng.standard_normal((E, E), np.float32) * 0.02,
        "bv": rng.standard_normal((E,), np.float32) * 0.02,
    }
    o1, o2 = kernel(**ins)
    print("ok", o1.shape, o2.shape, o1.dtype)


# revision 15
# speedup vs baseline: 1.2516x; 1.0818x over previous
"""Bass/Trainium2 kernel for blockwise cross-attention.

Math (per batch element b, per 16-row block):
  out1 = softmax(q1 k2^T / sqrt(E)) @ v2,  out2 = softmax(q2 k1^T / sqrt(E)) @ v1
with q = x Wq^T + bq etc.  Since softmax is shift-invariant along the key
axis, the q-side bias terms drop and
  softmax(q1 k2^T / s) == softmax(x1 A x2^T + 1 (x2 c)^T)
with A = Wq^T Wk / s and c = Wk^T bq / s precomputed on the host.  This
replaces 6 big projections with 4 (z = x A^T fused for both q&k roles, plus
v' = x Wv^T).  The v bias folds in exactly because softmax rows sum to 1.

Sharding: pure data-parallel — batch B=8, one batch element per NeuronCore.

Device flow per core (S=4096 rows, E=512), bf16 matmuls / fp32 softmax:
  - x^T tiles [128e, 512rows] per 512-row group (host pre-transposes, bf16)
  - z^T = A x^T via stationary A^T chunks; t = x c via stationary c chunks;
    v' = x Wv^T natural via stationary x^T chunks; v bias bv added during the
    PSUM->SBUF copy (DVE tensor_tensor with a broadcast bv tile)
  - scores window [128q,128k]: 4 e-chunk matmuls + one K=9 matmul that adds
    both the off-block -100 mask (rank 9: -100*1x1 + 100*sum u_b x u_b) and
    the key-side bias t[k] (folded into the rank-1 row as t[k]-100)
  - softmax: ACT Exp with fused accum row-sum (off-block entries exp to 0,
    so no explicit mask or max-subtraction is needed; logits are O(1)),
    DVE reciprocal + per-row scale -> bf16
  - attn^T: single DVE 32x32-block transpose == exact transpose of the
    block-diagonal attn (16-blocks lie inside diagonal 32-blocks; off-diag
    32-blocks are exactly 0)
  - out = attnT.T @ v' single K=128 matmul -> PSUM -> copy -> DMA out fp32
"""

import math
import sys

if "/opt/trn_rl_repo" not in sys.path:
    sys.path.insert(0, "/opt/trn_rl_repo")

import numpy as np
import ml_dtypes

BF16 = ml_dtypes.bfloat16
MASK_C = 100.0  # off-block logit penalty; exp(x - 100) flushes to 0 in fp32
BLOCK = 16  # attention block size (ceil(S**(2/3)) blocks => 16 for S=4096)


def _build_nc(S: int, E: int):
    from contextlib import ExitStack

    import concourse.bass as bass
    import concourse.tile as tile
    from concourse import bacc, mybir

    f32 = mybir.dt.float32
    bf16 = mybir.dt.bfloat16
    P = 128
    GROUP = 512  # rows per group
    G = S // GROUP
    NCH = E // P  # e-chunks (4)
    NW = GROUP // P  # windows per group (4)
    NB = P // BLOCK  # 16-blocks per window (8)
    assert S % GROUP == 0 and E == 512

    nc = bacc.Bacc("TRN2", debug=False)

    x_dram = [
        nc.dram_tensor("x1t", [E, S], bf16, kind="ExternalInput").ap(),
        nc.dram_tensor("x2t", [E, S], bf16, kind="ExternalInput").ap(),
    ]
    at_dram = nc.dram_tensor("at", [E, E], bf16, kind="ExternalInput").ap()
    wvt_dram = nc.dram_tensor("wvt", [E, E], bf16, kind="ExternalInput").ap()
    ml9_dram = nc.dram_tensor("ml9", [2 + NB, P], bf16, kind="ExternalInput").ap()
    # per-(state, group) rank-10 k-side rows: row0 = t = x@c (host-computed),
    # row1 = -C, rows 2..9 = C*u_b
    r5_dram = nc.dram_tensor(
        "r5all", [2, G, 2 + NB, GROUP], bf16, kind="ExternalInput"
    ).ap()
    bvb_dram = nc.dram_tensor("bvb", [P, E], f32, kind="ExternalInput").ap()
    out_dram = [
        nc.dram_tensor("out1", [S, E], f32, kind="ExternalOutput").ap(),
        nc.dram_tensor("out2", [S, E], f32, kind="ExternalOutput").ap(),
    ]

    Exp = mybir.ActivationFunctionType.Exp

    with ExitStack() as ctx:
        tc = ctx.enter_context(tile.TileContext(nc))

        consts = ctx.enter_context(tc.tile_pool(name="consts", bufs=1))
        xt_pool = ctx.enter_context(tc.tile_pool(name="xt", bufs=2))
        z_pool = ctx.enter_context(tc.tile_pool(name="z", bufs=2))
        v_pool = ctx.enter_context(tc.tile_pool(name="v", bufs=2))
        r5_pool = ctx.enter_context(tc.tile_pool(name="r5", bufs=2))
        sm_pool = ctx.enter_context(tc.tile_pool(name="sm", bufs=3))
        o_pool = ctx.enter_context(tc.tile_pool(name="o", bufs=3))
        psA = ctx.enter_context(tc.tile_pool(name="psA", bufs=3, space="PSUM"))
        psS = ctx.enter_context(tc.tile_pool(name="psS", bufs=3, space="PSUM"))
        psO = ctx.enter_context(tc.tile_pool(name="psO", bufs=2, space="PSUM"))

        # --- constants (batched DMAs) ---
        at_t = consts.tile([P, NCH * E], bf16, name="att", tag="att")
        nc.sync.dma_start(
            at_t.rearrange("p (c e) -> p c e", c=NCH),
            at_dram.rearrange("(c p) e -> p c e", p=P),
        )
        wv_t = consts.tile([P, NCH * E], bf16, name="wvt", tag="wvt")
        nc.sync.dma_start(
            wv_t.rearrange("p (c e) -> p c e", c=NCH),
            wvt_dram.rearrange("(c p) e -> p c e", p=P),
        )
        ml9_t = consts.tile([2 + NB, P], bf16, name="ml9", tag="ml9")
        nc.sync.dma_start(ml9_t[:], ml9_dram[:])
        bvb_t = consts.tile([P, E], f32, name="bvb", tag="bvb")
        nc.sync.dma_start(bvb_t[:], bvb_dram[:])

        def at_c(c):  # A^T chunk c: [128 e_in, 512 e_out]
            return at_t[:, c * E : (c + 1) * E]

        def wv_c(c):
            return wv_t[:, c * E : (c + 1) * E]

        # --- main loop over 512-row groups ---
        for g in range(G):
            r0 = g * GROUP
            xt = {}
            zt = {}
            vt = {}
            r5 = {}
            for s in range(2):
                x_tl = xt_pool.tile([P, NCH * GROUP], bf16, name=f"xt{s}", tag=f"xt{s}")
                nc.sync.dma_start(
                    x_tl.rearrange("p (c r) -> p c r", c=NCH),
                    x_dram[s].rearrange("(c p) s -> p c s", p=P)[:, :, r0 : r0 + GROUP],
                )
                xt[s] = x_tl

            def xt_c(s, c):  # x^T chunk c: [128 e_in, 512 rows]
                return xt[s][:, c * GROUP : (c + 1) * GROUP]

            for s in range(2):
                # scores-bias rhs tile [10, GROUP] — fully host-prepared
                r5_tl = r5_pool.tile([2 + NB, GROUP], bf16, name=f"r5{s}", tag=f"r5{s}")
                nc.sync.dma_start(r5_tl[:], r5_dram[s, g])
                r5[s] = r5_tl

                # z_s^T m-chunk [128 e_out, GROUP rows]
                for m in range(NCH):
                    z_ps = psA.tile([P, GROUP], f32, name="zps", tag="psA")
                    for c in range(NCH):
                        nc.tensor.matmul(
                            z_ps[:], at_c(c)[:, m * P : (m + 1) * P], xt_c(s, c),
                            start=(c == 0), stop=(c == NCH - 1),
                        )
                    z_sb = z_pool.tile([P, GROUP], bf16, name=f"zsb{s}{m}", tag=f"zsb{s}{m}")
                    nc.scalar.copy(z_sb[:], z_ps[:])
                    zt[s, m] = z_sb

                # v'_s r-chunk [128 rows, E] = x @ Wv^T ; + bv during copy
                for r in range(NW):
                    v_ps = psA.tile([P, E], f32, name="vps", tag="psA")
                    for c in range(NCH):
                        nc.tensor.matmul(
                            v_ps[:], xt_c(s, c)[:, r * P : (r + 1) * P], wv_c(c),
                            start=(c == 0), stop=(c == NCH - 1),
                        )
                    v_sb = v_pool.tile([P, E], bf16, name=f"vsb{s}{r}", tag=f"vsb{s}{r}")
                    nc.vector.tensor_add(v_sb[:], v_ps[:], bvb_t[:])
                    vt[s, r] = v_sb

            # --- attention windows ---
            for w in range(NW):
                ws = slice(w * P, (w + 1) * P)
                for qs, ks in ((0, 1), (1, 0)):
                    s_ps = psS.tile([P, P], f32, name="sps", tag="psS")
                    for m in range(NCH):
                        nc.tensor.matmul(
                            s_ps[:], xt_c(qs, m)[:, ws], zt[ks, m][:, ws],
                            start=(m == 0), stop=False,
                        )
                    # + mask (-C off-block) + t_ks[k]: rank-10, K=10 matmul
                    nc.tensor.matmul(s_ps[:], ml9_t[:], r5[ks][:, ws], start=False, stop=True)

                    exp_sb = sm_pool.tile([P, P], f32, name="expsb", tag="expsb")
                    rsum = sm_pool.tile([P, 1], f32, name="rsum", tag="rsum")
                    nc.scalar.activation(exp_sb[:], s_ps[:], Exp, accum_out=rsum[:])
                    rcp = sm_pool.tile([P, 1], f32, name="rcp", tag="rcp")
                    nc.vector.reciprocal(rcp[:], rsum[:])
                    attn = sm_pool.tile([P, P], bf16, name="attn", tag="attn")
                    nc.vector.tensor_scalar_mul(attn[:], exp_sb[:], rcp[:])
                    attnT = sm_pool.tile([P, P], bf16, name="attnT", tag="attnT")
                    nc.vector.transpose(attnT[:], attn[:])

                    o_ps = psO.tile([P, E], f32, name="ops", tag="psO")
                    nc.tensor.matmul(o_ps[:], attnT[:], vt[ks, w][:], start=True, stop=True)
                    o_sb = o_pool.tile([P, E], f32, name=f"osb{qs}", tag=f"osb{qs}")
                    if w % 2 == 0:
                        nc.scalar.copy(o_sb[:], o_ps[:])
                    else:
                        nc.vector.tensor_copy(o_sb[:], o_ps[:])
                    nc.gpsimd.dma_start(out_dram[qs][r0 + w * P : r0 + (w + 1) * P, :], o_sb[:])

    nc.compile()
    return nc


def _host_inputs(state1, state2, Wq, bq, Wk, bk, Wv, bv, S, E):
    """Build the per-core common (weight) arrays + per-core x arrays."""
    P = 128
    GROUP = 512
    NCH = E // P
    NB = P // BLOCK
    G = S // GROUP
    scale = math.sqrt(E)
    Wq64 = np.asarray(Wq, np.float64)
    Wk64 = np.asarray(Wk, np.float64)
    # A = Wq^T Wk / scale ; device needs A^T = Wk^T Wq / scale  [e_in, e_out]
    at = (Wk64.T @ Wq64 / scale).astype(BF16)
    cvec = (Wk64.T @ np.asarray(bq, np.float64) / scale).astype(np.float32)  # [E]
    wvt = np.ascontiguousarray(np.asarray(Wv, np.float32).T).astype(BF16)
    # rank-10 q-side factor: row0/row1 = 1, rows 2..9 = u_b
    idx = np.arange(P)
    ml9 = np.zeros((2 + NB, P), BF16)
    ml9[0, :] = 1.0
    ml9[1, :] = 1.0
    for b in range(NB):
        ml9[2 + b, :] = (idx // BLOCK == b).astype(np.float32)
    bvb = np.broadcast_to(np.asarray(bv, np.float32).reshape(1, E), (P, E))
    common = {
        "at": np.ascontiguousarray(at),
        "wvt": wvt,
        "ml9": ml9,
        "bvb": np.ascontiguousarray(bvb),
    }
    # k-side rank-10 rows, const across groups except row0 = t = x@c
    kidx = np.arange(GROUP) % P
    r5_const = np.zeros((2 + NB, GROUP), np.float32)
    r5_const[1, :] = -MASK_C
    for b in range(NB):
        r5_const[2 + b, :] = MASK_C * (kidx // BLOCK == b)
    x1 = np.asarray(state1, np.float32)
    x2 = np.asarray(state2, np.float32)
    B = x1.shape[0]
    per_core = []
    for b in range(B):
        r5all = np.broadcast_to(r5_const, (2, G, 2 + NB, GROUP)).copy()
        r5all[0, :, 0, :] = (x1[b] @ cvec).reshape(G, GROUP)
        r5all[1, :, 0, :] = (x2[b] @ cvec).reshape(G, GROUP)
        per_core.append(
            {
                "x1t": np.ascontiguousarray(x1[b].T).astype(BF16),
                "x2t": np.ascontiguousarray(x2[b].T).astype(BF16),
                "r5all": r5all.astype(BF16),
                **common,
            }
        )
    return per_core


_NC_CACHE = {}


def _get_nc(S, E):
    key = (S, E)
    if key not in _NC_CACHE:
        _NC_CACHE[key] = _build_nc(S, E)
    return _NC_CACHE[key]


def kernel(state1, state2, Wq, bq, Wk, bk, Wv, bv):
    from concourse.bass_utils import run_bass_kernel_spmd

    state1 = np.asarray(state1)
    B, S, E = state1.shape
    assert (B, S, E) == (8, 4096, 512), (B, S, E)

    nc = _get_nc(S, E)
    in_maps = _host_inputs(state1, state2, Wq, bq, Wk, bk, Wv, bv, S, E)
    res = run_bass_kernel_spmd(nc, in_maps, list(range(B)))
    out1 = np.stack([res.results[b]["out1"] for b in range(B)])
    out2 = np.stack([res.results[b]["out2"] for b in range(B)])
    return out1, out2


if __name__ == "__main__":
    rng = np.random.default_rng(0)
    B, S, E = 8, 4096, 512
    ins = {
        "state1": rng.standard_normal((B, S, E), np.float32),
        "state2": rng.standard_normal((B, S, E), np.float32),
        "Wq": rng.standard_normal((E, E), np.float32) * 0.02,
        "bq": rng.standard_normal((E,), np.float32) * 0.02,
        "Wk": rng.standard_normal((E, E), np.float32) * 0.02,
        "bk": rng.standard_normal((E,), np.float32) * 0.02,
        "Wv": rng.standard_normal((E, E), np.float32) * 0.02,
        "bv": rng.standard_normal((E,), np.float32) * 0.02,
    }
    o1, o2 = kernel(**ins)
    print("ok", o1.shape, o2.shape, o1.dtype)


# revision 18
# speedup vs baseline: 1.2537x; 1.0017x over previous
"""Bass/Trainium2 kernel for blockwise cross-attention.

Math (per batch element b, per 16-row block):
  out1 = softmax(q1 k2^T / sqrt(E)) @ v2,  out2 = softmax(q2 k1^T / sqrt(E)) @ v1
with q = x Wq^T + bq etc.  Since softmax is shift-invariant along the key
axis, the q-side bias terms drop and
  softmax(q1 k2^T / s) == softmax(x1 A x2^T + 1 (x2 c)^T)
with A = Wq^T Wk / s and c = Wk^T bq / s precomputed on the host.  This
replaces 6 big projections with 4 (z = x A^T fused for both q&k roles, plus
v' = x Wv^T).  The v bias folds in exactly because softmax rows sum to 1.

Sharding: pure data-parallel — batch B=8, one batch element per NeuronCore.

Device flow per core (S=4096 rows, E=512), bf16 matmuls / fp32 softmax:
  - x^T tiles [128e, 512rows] per 512-row group (host pre-transposes, bf16)
  - z^T = A x^T via stationary A^T chunks; t = x c via stationary c chunks;
    v' = x Wv^T natural via stationary x^T chunks; v bias bv added during the
    PSUM->SBUF copy (DVE tensor_tensor with a broadcast bv tile)
  - scores window [128q,128k]: 4 e-chunk matmuls + one K=9 matmul that adds
    both the off-block -100 mask (rank 9: -100*1x1 + 100*sum u_b x u_b) and
    the key-side bias t[k] (folded into the rank-1 row as t[k]-100)
  - softmax: ACT Exp with fused accum row-sum (off-block entries exp to 0,
    so no explicit mask or max-subtraction is needed; logits are O(1)),
    DVE reciprocal + per-row scale -> bf16
  - attn^T: single DVE 32x32-block transpose == exact transpose of the
    block-diagonal attn (16-blocks lie inside diagonal 32-blocks; off-diag
    32-blocks are exactly 0)
  - out = attnT.T @ v' single K=128 matmul -> PSUM -> copy -> DMA out fp32
"""

import math
import sys

if "/opt/trn_rl_repo" not in sys.path:
    sys.path.insert(0, "/opt/trn_rl_repo")

import numpy as np
import ml_dtypes

BF16 = ml_dtypes.bfloat16
MASK_C = 100.0  # off-block logit penalty; exp(x - 100) flushes to 0 in fp32
BLOCK = 16  # attention block size (ceil(S**(2/3)) blocks => 16 for S=4096)


def _build_nc(S: int, E: int):
    from contextlib import ExitStack

    import concourse.bass as bass
    import concourse.tile as tile
    from concourse import bacc, mybir

    f32 = mybir.dt.float32
    bf16 = mybir.dt.bfloat16
    P = 128
    GROUP = 512  # rows per group
    G = S // GROUP
    NCH = E // P  # e-chunks (4)
    NW = GROUP // P  # windows per group (4)
    NB = P // BLOCK  # 16-blocks per window (8)
    assert S % GROUP == 0 and E == 512

    nc = bacc.Bacc("TRN2", debug=False)

    x_dram = [
        nc.dram_tensor("x1t", [E, S], bf16, kind="ExternalInput").ap(),
        nc.dram_tensor("x2t", [E, S], bf16, kind="ExternalInput").ap(),
    ]
    at_dram = nc.dram_tensor("at", [E, E], bf16, kind="ExternalInput").ap()
    wvt_dram = nc.dram_tensor("wvt", [E, E], bf16, kind="ExternalInput").ap()
    ml9_dram = nc.dram_tensor("ml9", [2 + NB, P], bf16, kind="ExternalInput").ap()
    # per-(state, group) rank-10 k-side rows: row0 = t = x@c (host-computed),
    # row1 = -C, rows 2..9 = C*u_b
    r5_dram = nc.dram_tensor(
        "r5all", [2, G, 2 + NB, GROUP], bf16, kind="ExternalInput"
    ).ap()
    bvb_dram = nc.dram_tensor("bvb", [P, E], f32, kind="ExternalInput").ap()
    out_dram = [
        nc.dram_tensor("out1", [S, E], f32, kind="ExternalOutput").ap(),
        nc.dram_tensor("out2", [S, E], f32, kind="ExternalOutput").ap(),
    ]

    Exp = mybir.ActivationFunctionType.Exp

    with ExitStack() as ctx:
        tc = ctx.enter_context(tile.TileContext(nc))

        consts = ctx.enter_context(tc.tile_pool(name="consts", bufs=1))
        xt_pool = ctx.enter_context(tc.tile_pool(name="xt", bufs=2))
        z_pool = ctx.enter_context(tc.tile_pool(name="z", bufs=2))
        v_pool = ctx.enter_context(tc.tile_pool(name="v", bufs=2))
        r5_pool = ctx.enter_context(tc.tile_pool(name="r5", bufs=2))
        sm_pool = ctx.enter_context(tc.tile_pool(name="sm", bufs=3))
        o_pool = ctx.enter_context(tc.tile_pool(name="o", bufs=3))
        psA = ctx.enter_context(tc.tile_pool(name="psA", bufs=4, space="PSUM"))
        psS = ctx.enter_context(tc.tile_pool(name="psS", bufs=2, space="PSUM"))
        psO = ctx.enter_context(tc.tile_pool(name="psO", bufs=2, space="PSUM"))

        # --- PE warmup: dependency-free matmuls on a zeroed scratch tile so
        # the HAM clock-gate reaches K=8/8 before the first real matmul ---
        wu_t = consts.tile([P, P], bf16, name="wut", tag="wut")
        nc.gpsimd.memset(wu_t[:], 0.0)
        wu_ps = psS.tile([P, P], f32, name="wups", tag="psS")
        for _ in range(28):
            nc.tensor.matmul(wu_ps[:], wu_t[:], wu_t[:], start=True, stop=True)

        # --- constants (batched DMAs; at first — first z matmul needs it) ---
        at_t = consts.tile([P, NCH * E], bf16, name="att", tag="att")
        nc.sync.dma_start(
            at_t.rearrange("p (c e) -> p c e", c=NCH),
            at_dram.rearrange("(c p) e -> p c e", p=P),
        )
        wv_t = consts.tile([P, NCH * E], bf16, name="wvt", tag="wvt")
        nc.scalar.dma_start(
            wv_t.rearrange("p (c e) -> p c e", c=NCH),
            wvt_dram.rearrange("(c p) e -> p c e", p=P),
        )
        ml9_t = consts.tile([2 + NB, P], bf16, name="ml9", tag="ml9")
        nc.scalar.dma_start(ml9_t[:], ml9_dram[:])
        bvb_t = consts.tile([P, E], f32, name="bvb", tag="bvb")
        nc.scalar.dma_start(bvb_t[:], bvb_dram[:])

        def at_c(c):  # A^T chunk c: [128 e_in, 512 e_out]
            return at_t[:, c * E : (c + 1) * E]

        def wv_c(c):
            return wv_t[:, c * E : (c + 1) * E]

        # --- main loop over 512-row groups ---
        for g in range(G):
            r0 = g * GROUP
            xt = {}
            zt = {}
            vt = {}
            r5 = {}
            for s in range(2):
                x_tl = xt_pool.tile([P, NCH * GROUP], bf16, name=f"xt{s}", tag=f"xt{s}")
                nc.sync.dma_start(
                    x_tl.rearrange("p (c r) -> p c r", c=NCH),
                    x_dram[s].rearrange("(c p) s -> p c s", p=P)[:, :, r0 : r0 + GROUP],
                )
                xt[s] = x_tl

            def xt_c(s, c):  # x^T chunk c: [128 e_in, 512 rows]
                return xt[s][:, c * GROUP : (c + 1) * GROUP]

            for s in range(2):
                # scores-bias rhs tile [10, GROUP] — fully host-prepared
                r5_tl = r5_pool.tile([2 + NB, GROUP], bf16, name=f"r5{s}", tag=f"r5{s}")
                nc.gpsimd.dma_start(r5_tl[:], r5_dram[s, g])
                r5[s] = r5_tl

                # z_s^T m-chunk [128 e_out, GROUP rows]
                for m in range(NCH):
                    z_ps = psA.tile([P, GROUP], f32, name="zps", tag="psA")
                    for c in range(NCH):
                        nc.tensor.matmul(
                            z_ps[:], at_c(c)[:, m * P : (m + 1) * P], xt_c(s, c),
                            start=(c == 0), stop=(c == NCH - 1),
                        )
                    z_sb = z_pool.tile([P, GROUP], bf16, name=f"zsb{s}{m}", tag=f"zsb{s}{m}")
                    nc.scalar.copy(z_sb[:], z_ps[:])
                    zt[s, m] = z_sb

                # v'_s r-chunk [128 rows, E] = x @ Wv^T ; + bv during copy
                for r in range(NW):
                    v_ps = psA.tile([P, E], f32, name="vps", tag="psA")
                    for c in range(NCH):
                        nc.tensor.matmul(
                            v_ps[:], xt_c(s, c)[:, r * P : (r + 1) * P], wv_c(c),
                            start=(c == 0), stop=(c == NCH - 1),
                        )
                    v_sb = v_pool.tile([P, E], bf16, name=f"vsb{s}{r}", tag=f"vsb{s}{r}")
                    nc.vector.tensor_add(v_sb[:], v_ps[:], bvb_t[:])
                    vt[s, r] = v_sb

            # --- attention windows ---
            for w in range(NW):
                ws = slice(w * P, (w + 1) * P)
                for qs, ks in ((0, 1), (1, 0)):
                    s_ps = psS.tile([P, P], f32, name="sps", tag="psS")
                    for m in range(NCH):
                        nc.tensor.matmul(
                            s_ps[:], xt_c(qs, m)[:, ws], zt[ks, m][:, ws],
                            start=(m == 0), stop=False,
                        )
                    # + mask (-C off-block) + t_ks[k]: rank-10, K=10 matmul
                    nc.tensor.matmul(s_ps[:], ml9_t[:], r5[ks][:, ws], start=False, stop=True)

                    exp_sb = sm_pool.tile([P, P], f32, name="expsb", tag="expsb")
                    rsum = sm_pool.tile([P, 1], f32, name="rsum", tag="rsum")
                    nc.scalar.activation(exp_sb[:], s_ps[:], Exp, accum_out=rsum[:])
                    rcp = sm_pool.tile([P, 1], f32, name="rcp", tag="rcp")
                    nc.vector.reciprocal(rcp[:], rsum[:])
                    attn = sm_pool.tile([P, P], bf16, name="attn", tag="attn")
                    nc.vector.tensor_scalar_mul(attn[:], exp_sb[:], rcp[:])
                    attnT = sm_pool.tile([P, P], bf16, name="attnT", tag="attnT")
                    nc.vector.transpose(attnT[:], attn[:])

                    o_ps = psO.tile([P, E], f32, name="ops", tag="psO")
                    nc.tensor.matmul(o_ps[:], attnT[:], vt[ks, w][:], start=True, stop=True)
                    o_sb = o_pool.tile([P, E], f32, name=f"osb{qs}", tag=f"osb{qs}")
                    if w % 2 == 0:
                        nc.scalar.copy(o_sb[:], o_ps[:])
                    else:
                        nc.vector.tensor_copy(o_sb[:], o_ps[:])
                    nc.gpsimd.dma_start(out_dram[qs][r0 + w * P : r0 + (w + 1) * P, :], o_sb[:])

    nc.compile()
    return nc


def _host_inputs(state1, state2, Wq, bq, Wk, bk, Wv, bv, S, E):
    """Build the per-core common (weight) arrays + per-core x arrays."""
    P = 128
    GROUP = 512
    NCH = E // P
    NB = P // BLOCK
    G = S // GROUP
    scale = math.sqrt(E)
    Wq64 = np.asarray(Wq, np.float64)
    Wk64 = np.asarray(Wk, np.float64)
    # A = Wq^T Wk / scale ; device needs A^T = Wk^T Wq / scale  [e_in, e_out]
    at = (Wk64.T @ Wq64 / scale).astype(BF16)
    cvec = (Wk64.T @ np.asarray(bq, np.float64) / scale).astype(np.float32)  # [E]
    wvt = np.ascontiguousarray(np.asarray(Wv, np.float32).T).astype(BF16)
    # rank-10 q-side factor: row0/row1 = 1, rows 2..9 = u_b
    idx = np.arange(P)
    ml9 = np.zeros((2 + NB, P), BF16)
    ml9[0, :] = 1.0
    ml9[1, :] = 1.0
    for b in range(NB):
        ml9[2 + b, :] = (idx // BLOCK == b).astype(np.float32)
    bvb = np.broadcast_to(np.asarray(bv, np.float32).reshape(1, E), (P, E))
    common = {
        "at": np.ascontiguousarray(at),
        "wvt": wvt,
        "ml9": ml9,
        "bvb": np.ascontiguousarray(bvb),
    }
    # k-side rank-10 rows, const across groups except row0 = t = x@c
    kidx = np.arange(GROUP) % P
    r5_const = np.zeros((2 + NB, GROUP), np.float32)
    r5_const[1, :] = -MASK_C
    for b in range(NB):
        r5_const[2 + b, :] = MASK_C * (kidx // BLOCK == b)
    x1 = np.asarray(state1, np.float32)
    x2 = np.asarray(state2, np.float32)
    B = x1.shape[0]
    per_core = []
    for b in range(B):
        r5all = np.broadcast_to(r5_const, (2, G, 2 + NB, GROUP)).copy()
        r5all[0, :, 0, :] = (x1[b] @ cvec).reshape(G, GROUP)
        r5all[1, :, 0, :] = (x2[b] @ cvec).reshape(G, GROUP)
        per_core.append(
            {
                "x1t": np.ascontiguousarray(x1[b].T).astype(BF16),
                "x2t": np.ascontiguousarray(x2[b].T).astype(BF16),
                "r5all": r5all.astype(BF16),
                **common,
            }
        )
    return per_core


_NC_CACHE = {}


def _get_nc(S, E):
    key = (S, E)
    if key not in _NC_CACHE:
        _NC_CACHE[key] = _build_nc(S, E)
    return _NC_CACHE[key]


def kernel(state1, state2, Wq, bq, Wk, bk, Wv, bv):
    from concourse.bass_utils import run_bass_kernel_spmd

    state1 = np.asarray(state1)
    B, S, E = state1.shape
    assert (B, S, E) == (8, 4096, 512), (B, S, E)

    nc = _get_nc(S, E)
    in_maps = _host_inputs(state1, state2, Wq, bq, Wk, bk, Wv, bv, S, E)
    res = run_bass_kernel_spmd(nc, in_maps, list(range(B)))
    out1 = np.stack([res.results[b]["out1"] for b in range(B)])
    out2 = np.stack([res.results[b]["out2"] for b in range(B)])
    return out1, out2


if __name__ == "__main__":
    rng = np.random.default_rng(0)
    B, S, E = 8, 4096, 512
    ins = {
        "state1": rng.standard_normal((B, S, E), np.float32),
        "state2": rng.standard_normal((B, S, E), np.float32),
        "Wq": rng.standard_normal((E, E), np.float32) * 0.02,
        "bq": rng.standard_normal((E,), np.float32) * 0.02,
        "Wk": rng.standard_normal((E, E), np.float32) * 0.02,
        "bk": rng.standard_normal((E,), np.float32) * 0.02,
        "Wv": rng.standard_normal((E, E), np.float32) * 0.02,
        "bv": rng.standard_normal((E,), np.float32) * 0.02,
    }
    o1, o2 = kernel(**ins)
    print("ok", o1.shape, o2.shape, o1.dtype)
